# revision 10
# baseline (speedup 1.0000x reference)
"""Trainium2 Bass kernel for the 8-level butterfly layer.

Contract: kernel(**inputs) takes FULL unsharded numpy inputs
(in_data [512,4096], W_in [16,64], b_in [64], W_lvl [510,2,64,64],
b_lvl [510,64], Fea [256,64,16]) and returns the FULL output
(512, 4096, 1) float32.

Strategy: pure data parallelism over batch (64 rows per core, 8 cores),
butterfly filters replicated. Per core, each level is a set of K=128
contraction matmuls in bf16. The t-parity split needed by the next
level's pair concatenation is produced by the matmul itself via
column-tiled PE matmuls (even-t columns -> PSUM partitions 0:64, odd-t
-> 64:128), so every PSUM->SBUF relu+bias+cast runs on all 128
partitions with no partition shifts.

Activation layout per level L ("pair format"), one SBUF tensor
R_L [128, 8192] bf16: box c of level L occupies columns
[c*N, (c+1)*N), N = 2^(13-L); partition (s*64 + ch) holds channel ch of
position t with parity s; column within the box block is (t//2)*64 + b.
"""

import numpy as np
import ml_dtypes

import concourse.bass as bass
import concourse.mybir as mybir
import concourse.tile as tile
from concourse import bacc

NCORES = 8
B = 512
BC = B // NCORES  # 64 batch rows per core
NLVL = 8
C = 64
FIN = 16
FOUT = 16
KTOT = 256  # 2**NLVL
INS = 4096

BF16 = mybir.dt.bfloat16
F32 = mybir.dt.float32

_CACHE: dict = {}


def _bf16(a: np.ndarray) -> np.ndarray:
    return np.ascontiguousarray(a.astype(np.float32)).astype(ml_dtypes.bfloat16)


def pack_shared(W_in, b_in, W_lvl, b_lvl, Fea) -> dict:
    """Host-side repacking of the replicated filter tensors."""
    W_in = np.asarray(W_in, np.float32)
    b_in = np.asarray(b_in, np.float32)
    W_lvl = np.asarray(W_lvl, np.float32)
    b_lvl = np.asarray(b_lvl, np.float32)
    Fea = np.asarray(Fea, np.float32)

    # Levels 1..7 weights: boxes are W_lvl[0:254] in level-major order.
    # [254, 2, 64, 64] -> [254, 128, 64] (row = s*64+c_in) -> [128, 254*64]
    wmain = W_lvl[0:254].reshape(254, 128, 64).transpose(1, 0, 2).reshape(128, 254 * 64)

    # Level 8 weights, pair-packed: pair p holds boxes 2p, 2p+1
    # (global idx 254+2p, 254+2p+1). [128, (p, j, m)] -> [128, 16384]
    w8 = W_lvl[254:510].reshape(128, 2, 128, 64).transpose(2, 0, 1, 3).reshape(128, 128 * 128)

    # Fea pair-packed blockdiag: [128 rows (j,ch), 128 pairs, 32]
    fea = np.zeros((128, 128, 32), np.float32)
    fea[0:64, :, 0:16] = Fea[0::2].transpose(1, 0, 2)
    fea[64:128, :, 16:32] = Fea[1::2].transpose(1, 0, 2)
    fea = fea.reshape(128, 128 * 32)

    # Biases (fp32): duplicated across partition halves for levels in..7,
    # pair-format for level 8.
    bin_h = np.concatenate([b_in, b_in]).reshape(128, 1)
    bd7 = np.concatenate([b_lvl[0:254], b_lvl[0:254]], axis=1).T.copy()  # [128, 254]
    bp8 = b_lvl[254:510].reshape(128, 2, 64).transpose(1, 2, 0).reshape(128, 128).copy()

    return {
        "win": _bf16(W_in),
        "wmain": _bf16(wmain),
        "w8": _bf16(w8),
        "fea": _bf16(fea),
        "bin": np.ascontiguousarray(bin_h, np.float32),
        "bd7": np.ascontiguousarray(bd7, np.float32),
        "bp8": np.ascontiguousarray(bp8, np.float32),
    }


def pack_x(x_shard: np.ndarray) -> np.ndarray:
    """[64, 4096] batch shard -> [16, 16384] bf16: [xe | xo], col = k'*64 + b."""
    xs = np.asarray(x_shard, np.float32).reshape(BC, KTOT, FIN)
    xe = xs[:, 0::2, :].transpose(2, 1, 0).reshape(FIN, 128 * BC)
    xo = xs[:, 1::2, :].transpose(2, 1, 0).reshape(FIN, 128 * BC)
    return _bf16(np.concatenate([xe, xo], axis=1))


def _build_module():
    nc = bacc.Bacc("TRN2", target_bir_lowering=False, debug=False)

    xt = nc.dram_tensor("xt", [FIN, 2 * 128 * BC], BF16, kind="ExternalInput")
    win = nc.dram_tensor("win", [FIN, C], BF16, kind="ExternalInput")
    wmain = nc.dram_tensor("wmain", [128, 254 * 64], BF16, kind="ExternalInput")
    w8 = nc.dram_tensor("w8", [128, 128 * 128], BF16, kind="ExternalInput")
    fea = nc.dram_tensor("fea", [128, 128 * 32], BF16, kind="ExternalInput")
    bin_t = nc.dram_tensor("bin", [128, 1], F32, kind="ExternalInput")
    bd7 = nc.dram_tensor("bd7", [128, 254], F32, kind="ExternalInput")
    bp8 = nc.dram_tensor("bp8", [128, 128], F32, kind="ExternalInput")
    out = nc.dram_tensor("out", [BC, KTOT * FOUT], F32, kind="ExternalOutput")

    relu = mybir.ActivationFunctionType.Relu
    evac_cnt = 0

    def evac(dst, src, bias_ap):
        nonlocal evac_cnt
        if evac_cnt % 2 == 0:
            nc.scalar.activation(dst, src, relu, bias=bias_ap)
        else:
            nc.vector.tensor_scalar(
                dst, src, bias_ap, 0.0,
                op0=mybir.AluOpType.add, op1=mybir.AluOpType.max,
            )
        evac_cnt += 1

    with tile.TileContext(nc) as tc:
        with (
            tc.tile_pool(name="wp", bufs=1) as wp,
            tc.tile_pool(name="xp", bufs=1) as xp,
            tc.tile_pool(name="rp", bufs=3) as rp,
            tc.tile_pool(name="op", bufs=2) as op,
            tc.tile_pool(name="ps", bufs=4, space="PSUM") as ps,
            tc.tile_pool(name="po", bufs=2, space="PSUM") as po,
        ):
            x_sb = xp.tile([FIN, 2 * 128 * BC], BF16, tag="x")
            nc.sync.dma_start(x_sb[:, :], xt.ap())
            win_sb = wp.tile([FIN, C], BF16, tag="win")
            nc.sync.dma_start(win_sb[:, :], win.ap())
            bin_sb = wp.tile([128, 1], F32, tag="bin")
            nc.sync.dma_start(bin_sb[:, :], bin_t.ap())
            bd7_sb = wp.tile([128, 254], F32, tag="bd7")
            nc.sync.dma_start(bd7_sb[:, :], bd7.ap())
            bp8_sb = wp.tile([128, 128], F32, tag="bp8")
            nc.sync.dma_start(bp8_sb[:, :], bp8.ap())

            wt = {}
            for lv in range(1, 8):
                nb = 2 ** lv  # boxes at this level
                wt[lv] = wp.tile([128, nb * 64], BF16, tag=f"wt{lv}", name=f"wt{lv}")
                c0 = (2 ** lv - 2) * 64
                nc.sync.dma_start(wt[lv][:, :], wmain.ap()[:, c0:c0 + nb * 64])
            w8_sb = []
            for h in range(2):
                t = wp.tile([128, 8192], BF16, tag=f"w8{h}", name=f"w8{h}")
                nc.sync.dma_start(t[:, :], w8.ap()[:, h * 8192:(h + 1) * 8192])
                w8_sb.append(t)
            fea_sb = wp.tile([128, 128 * 32], BF16, tag="fea")
            nc.sync.dma_start(fea_sb[:, :], fea.ap())

            # ---- input interpolation: x [16,(k',b)] @ W_in -> R0
            R = rp.tile([128, 8192], BF16, tag="R")
            for j in range(16):
                pc = ps.tile([128, 512], F32, tag="ps")
                for q in range(2):
                    rhs = x_sb[:, q * 8192 + j * 512: q * 8192 + (j + 1) * 512]
                    nc.tensor.matmul(
                        pc[64 * q:64 * (q + 1), :], lhsT=win_sb[:, :], rhs=rhs,
                        start=True, stop=True, tile_position=(0, 64 * q),
                    )
                evac(R[:, j * 512:(j + 1) * 512], pc[:, :], bin_sb[:, :])

            # ---- butterfly levels 1..7
            for lv in range(1, 8):
                P = 2 ** (lv - 1)            # parent boxes at level lv-1
                Np = 2 ** (14 - lv)          # parent block columns
                Ncb = Np // 2                # child block columns
                S = min(512, Ncb)            # psum chunk columns
                TU = S // 64                 # t-pair units per chunk
                Rn = rp.tile([128, 8192], BF16, tag="R")
                for p in range(P):
                    pv = Rp_view = R[:, p * Np:(p + 1) * Np].rearrange(
                        "a (t2 two b) -> a t2 two b", two=2, b=64)
                    for cl in range(2):
                        box = 2 * p + cl
                        lhsT = wt[lv][:, box * 64:(box + 1) * 64]
                        for j in range(Ncb // S):
                            pc = ps.tile([128, 512], F32, tag="ps")
                            for q in range(2):
                                rhs = pv[:, j * TU:(j + 1) * TU, q, :]
                                nc.tensor.matmul(
                                    pc[64 * q:64 * (q + 1), 0:S], lhsT=lhsT, rhs=rhs,
                                    start=True, stop=True, tile_position=(0, 64 * q),
                                )
                            bc = 2 ** lv - 2 + box
                            evac(
                                Rn[:, box * Ncb + j * S: box * Ncb + (j + 1) * S],
                                pc[:, 0:S], bd7_sb[:, bc:bc + 1],
                            )
                R = Rn

            # ---- level 8: pair-packed, no parity split (V8 group tiles)
            Rn = rp.tile([128, 8192], BF16, tag="R")
            for p in range(128):
                pc = ps.tile([128, 512], F32, tag="ps")
                lhsT = w8_sb[p // 64][:, (p % 64) * 128:(p % 64) * 128 + 128]
                nc.tensor.matmul(
                    pc[:, 0:64], lhsT=lhsT, rhs=R[:, p * 64:(p + 1) * 64],
                    start=True, stop=True,
                )
                evac(Rn[:, p * 64:(p + 1) * 64], pc[:, 0:64], bp8_sb[:, p:p + 1])
            R = Rn

            # ---- output: y[b, 32p+m] = V8g[:, b] . Fea_pk[:, (p, m)]
            for bt in range(8):
                pco = po.tile([BC, 512], F32, tag="po")
                for g in range(16):
                    p = bt * 16 + g
                    nc.tensor.matmul(
                        pco[:, g * 32:(g + 1) * 32],
                        lhsT=R[:, p * 64:(p + 1) * 64],
                        rhs=fea_sb[:, p * 32:(p + 1) * 32],
                        start=True, stop=True,
                    )
                o_sb = op.tile([BC, 512], F32, tag="os")
                if bt % 2 == 0:
                    nc.scalar.copy(o_sb[:, :], pco[:, :])
                else:
                    nc.vector.tensor_copy(o_sb[:, :], pco[:, :])
                nc.sync.dma_start(out.ap()[:, bt * 512:(bt + 1) * 512], o_sb[:, :])

    nc.compile()
    return nc


def _make_runner(nc):
    """Cached jitted SPMD runner over the 8 cores (mirrors
    bass2jax.run_bass_via_pjrt, but reusable across calls and without
    donation — this kernel writes every output element)."""
    import jax

    from concourse.bass2jax import (
        _bass_exec_p,
        install_neuronx_cc_hook,
        partition_id_tensor,
    )
    from jax.experimental.shard_map import shard_map
    from jax.sharding import Mesh, PartitionSpec

    install_neuronx_cc_hook()

    partition_name = nc.partition_id_tensor.name if nc.partition_id_tensor else None
    in_names: list[str] = []
    out_names: list[str] = []
    out_avals = []
    zero_outs: list[np.ndarray] = []
    for alloc in nc.m.functions[0].allocations:
        if not isinstance(alloc, mybir.MemoryLocationSet):
            continue
        name = alloc.memorylocations[0].name
        if alloc.kind == "ExternalInput":
            if name != partition_name:
                in_names.append(name)
        elif alloc.kind == "ExternalOutput":
            shape = tuple(alloc.tensor_shape)
            dtype = mybir.dt.np(alloc.dtype)
            out_names.append(name)
            out_avals.append(jax.core.ShapedArray(shape, dtype))
            zero_outs.append(np.zeros(shape, dtype))
    n_params = len(in_names)
    all_names = in_names + out_names
    if partition_name is not None:
        all_names = all_names + [partition_name]

    def _body(*args):
        operands = list(args)
        if partition_name is not None:
            operands.append(partition_id_tensor())
        outs = _bass_exec_p.bind(
            *operands,
            out_avals=tuple(out_avals),
            in_names=tuple(all_names),
            out_names=tuple(out_names),
            lowering_input_output_aliases=(),
            sim_require_finite=True,
            sim_require_nnan=True,
            nc=nc,
        )
        return tuple(outs)

    devices = jax.devices()[:NCORES]
    mesh = Mesh(np.asarray(devices), ("core",))
    n_all = n_params + len(out_names)
    sharded = jax.jit(
        shard_map(
            _body, mesh=mesh,
            in_specs=(PartitionSpec("core"),) * n_all,
            out_specs=(PartitionSpec("core"),) * len(out_names),
            check_rep=False,
        ),
        keep_unused=True,
    )
    return {
        "fn": sharded,
        "in_names": in_names,
        "out_names": out_names,
        "out_avals": out_avals,
        "zero_outs": zero_outs,
    }


def _runner():
    if "nc" not in _CACHE:
        _CACHE["nc"] = _build_module()
    if "runner" not in _CACHE:
        _CACHE["runner"] = _make_runner(_CACHE["nc"])
    return _CACHE["runner"]


def _concat_args(in_maps):
    r = _runner()
    args = [
        np.concatenate([np.asarray(m[name]) for m in in_maps], axis=0)
        for name in r["in_names"]
    ]
    args += [
        np.zeros((NCORES * z.shape[0], *z.shape[1:]), z.dtype) for z in r["zero_outs"]
    ]
    return args


def kernel(**inputs) -> np.ndarray:
    r = _runner()
    shared = pack_shared(
        inputs["W_in"], inputs["b_in"], inputs["W_lvl"], inputs["b_lvl"], inputs["Fea"]
    )
    in_data = np.asarray(inputs["in_data"], np.float32)
    in_maps = []
    for c in range(NCORES):
        m = dict(shared)
        m["xt"] = pack_x(in_data[c * BC:(c + 1) * BC])
        in_maps.append(m)

    out_arrs = r["fn"](*_concat_args(in_maps))
    out = np.asarray(out_arrs[r["out_names"].index("out")])
    return out.reshape(B, KTOT * FOUT, 1).astype(np.float32)


# revision 12
# speedup vs baseline: 80.1452x; 80.1452x over previous
"""Trainium2 Bass kernel for the 8-level butterfly layer.

Contract: kernel(**inputs) takes FULL unsharded numpy inputs
(in_data [512,4096], W_in [16,64], b_in [64], W_lvl [510,2,64,64],
b_lvl [510,64], Fea [256,64,16]) and returns the FULL output
(512, 4096, 1) float32.

Strategy: pure data parallelism over batch (64 rows per core, 8 cores),
butterfly filters replicated. Per core, each level is a set of K=128
contraction matmuls in bf16. The t-parity split needed by the next
level's pair concatenation is produced by the matmul itself via
column-tiled PE matmuls (even-t columns -> PSUM partitions 0:64, odd-t
-> 64:128), so every PSUM->SBUF relu+bias+cast runs on all 128
partitions with no partition shifts.

Activation layout per level L ("pair format"), one SBUF tensor
R_L [128, 8192] bf16: box c of level L occupies columns
[c*N, (c+1)*N), N = 2^(13-L); partition (s*64 + ch) holds channel ch of
position t with parity s; column within the box block is (t//2)*64 + b.
"""

import numpy as np
import ml_dtypes

import concourse.bass as bass
import concourse.mybir as mybir
import concourse.tile as tile
from concourse import bacc

NCORES = 8
B = 512
BC = B // NCORES  # 64 batch rows per core
NLVL = 8
C = 64
FIN = 16
FOUT = 16
KTOT = 256  # 2**NLVL
INS = 4096

BF16 = mybir.dt.bfloat16
F32 = mybir.dt.float32

_CACHE: dict = {}


def _bf16(a: np.ndarray) -> np.ndarray:
    return np.ascontiguousarray(a.astype(np.float32)).astype(ml_dtypes.bfloat16)


def pack_shared(W_in, b_in, W_lvl, b_lvl, Fea) -> dict:
    """Host-side repacking of the replicated filter tensors."""
    W_in = np.asarray(W_in, np.float32)
    b_in = np.asarray(b_in, np.float32)
    W_lvl = np.asarray(W_lvl, np.float32)
    b_lvl = np.asarray(b_lvl, np.float32)
    Fea = np.asarray(Fea, np.float32)

    # Levels 1..7 weights: boxes are W_lvl[0:254] in level-major order.
    # [254, 2, 64, 64] -> [254, 128, 64] (row = s*64+c_in) -> [128, 254*64]
    wmain = W_lvl[0:254].reshape(254, 128, 64).transpose(1, 0, 2).reshape(128, 254 * 64)

    # Level 8 weights, pair-packed: pair p holds boxes 2p, 2p+1
    # (global idx 254+2p, 254+2p+1). [128, (p, j, m)] -> [128, 16384]
    w8 = W_lvl[254:510].reshape(128, 2, 128, 64).transpose(2, 0, 1, 3).reshape(128, 128 * 128)

    # Fea pair-packed blockdiag: [128 rows (j,ch), 128 pairs, 32]
    fea = np.zeros((128, 128, 32), np.float32)
    fea[0:64, :, 0:16] = Fea[0::2].transpose(1, 0, 2)
    fea[64:128, :, 16:32] = Fea[1::2].transpose(1, 0, 2)
    fea = fea.reshape(128, 128 * 32)

    # Biases (fp32): duplicated across partition halves for levels in..7,
    # pair-format for level 8.
    bin_h = np.concatenate([b_in, b_in]).reshape(128, 1)
    bd7 = np.concatenate([b_lvl[0:254], b_lvl[0:254]], axis=1).T.copy()  # [128, 254]
    bp8 = b_lvl[254:510].reshape(128, 2, 64).transpose(1, 2, 0).reshape(128, 128).copy()

    return {
        "win": _bf16(W_in),
        "wmain": _bf16(wmain),
        "w8": _bf16(w8),
        "fea": _bf16(fea),
        "bin": np.ascontiguousarray(bin_h, np.float32),
        "bd7": np.ascontiguousarray(bd7, np.float32),
        "bp8": np.ascontiguousarray(bp8, np.float32),
    }


def pack_x(x_shard: np.ndarray) -> np.ndarray:
    """[64, 4096] batch shard -> [16, 16384] bf16: [xe | xo], col = k'*64 + b."""
    xs = np.asarray(x_shard, np.float32).reshape(BC, KTOT, FIN)
    xe = xs[:, 0::2, :].transpose(2, 1, 0).reshape(FIN, 128 * BC)
    xo = xs[:, 1::2, :].transpose(2, 1, 0).reshape(FIN, 128 * BC)
    return _bf16(np.concatenate([xe, xo], axis=1))


def _build_module(loop_iters: int | None = None):
    """Build the bass module. loop_iters wraps the whole body in an
    on-device For_i loop (benchmarking only; graded path uses None)."""
    nc = bacc.Bacc("TRN2", target_bir_lowering=False, debug=False)

    xt = nc.dram_tensor("xt", [FIN, 2 * 128 * BC], BF16, kind="ExternalInput")
    win = nc.dram_tensor("win", [FIN, C], BF16, kind="ExternalInput")
    wmain = nc.dram_tensor("wmain", [128, 254 * 64], BF16, kind="ExternalInput")
    w8 = nc.dram_tensor("w8", [128, 128 * 128], BF16, kind="ExternalInput")
    fea = nc.dram_tensor("fea", [128, 128 * 32], BF16, kind="ExternalInput")
    bin_t = nc.dram_tensor("bin", [128, 1], F32, kind="ExternalInput")
    bd7 = nc.dram_tensor("bd7", [128, 254], F32, kind="ExternalInput")
    bp8 = nc.dram_tensor("bp8", [128, 128], F32, kind="ExternalInput")
    out = nc.dram_tensor("out", [BC, KTOT * FOUT], F32, kind="ExternalOutput")

    relu = mybir.ActivationFunctionType.Relu
    evac_cnt = 0

    def evac(dst, src, bias_ap):
        nonlocal evac_cnt
        if evac_cnt % 2 == 0:
            nc.scalar.activation(dst, src, relu, bias=bias_ap)
        else:
            nc.vector.tensor_scalar(
                dst, src, bias_ap, 0.0,
                op0=mybir.AluOpType.add, op1=mybir.AluOpType.max,
            )
        evac_cnt += 1

    import contextlib

    with tile.TileContext(nc) as tc:
        with (
            tc.tile_pool(name="wp", bufs=1) as wp,
            tc.tile_pool(name="xp", bufs=1) as xp,
            tc.tile_pool(name="rp", bufs=3) as rp,
            tc.tile_pool(name="op", bufs=2) as op,
            tc.tile_pool(name="ps", bufs=4, space="PSUM") as ps,
            tc.tile_pool(name="po", bufs=2, space="PSUM") as po,
            tc.For_i(0, loop_iters, 1) if loop_iters else contextlib.nullcontext(),
        ):
            x_sb = xp.tile([FIN, 2 * 128 * BC], BF16, tag="x")
            nc.sync.dma_start(x_sb[:, :], xt.ap())
            win_sb = wp.tile([FIN, C], BF16, tag="win")
            nc.sync.dma_start(win_sb[:, :], win.ap())
            bin_sb = wp.tile([128, 1], F32, tag="bin")
            nc.sync.dma_start(bin_sb[:, :], bin_t.ap())
            bd7_sb = wp.tile([128, 254], F32, tag="bd7")
            nc.sync.dma_start(bd7_sb[:, :], bd7.ap())
            bp8_sb = wp.tile([128, 128], F32, tag="bp8")
            nc.sync.dma_start(bp8_sb[:, :], bp8.ap())

            wt = {}
            for lv in range(1, 8):
                nb = 2 ** lv  # boxes at this level
                wt[lv] = wp.tile([128, nb * 64], BF16, tag=f"wt{lv}", name=f"wt{lv}")
                c0 = (2 ** lv - 2) * 64
                nc.sync.dma_start(wt[lv][:, :], wmain.ap()[:, c0:c0 + nb * 64])
            w8_sb = []
            for h in range(2):
                t = wp.tile([128, 8192], BF16, tag=f"w8{h}", name=f"w8{h}")
                nc.sync.dma_start(t[:, :], w8.ap()[:, h * 8192:(h + 1) * 8192])
                w8_sb.append(t)
            fea_sb = wp.tile([128, 128 * 32], BF16, tag="fea")
            nc.sync.dma_start(fea_sb[:, :], fea.ap())

            # ---- input interpolation: x [16,(k',b)] @ W_in -> R0
            R = rp.tile([128, 8192], BF16, tag="R")
            for j in range(16):
                pc = ps.tile([128, 512], F32, tag="ps")
                for q in range(2):
                    rhs = x_sb[:, q * 8192 + j * 512: q * 8192 + (j + 1) * 512]
                    nc.tensor.matmul(
                        pc[64 * q:64 * (q + 1), :], lhsT=win_sb[:, :], rhs=rhs,
                        start=True, stop=True, tile_position=(0, 64 * q),
                    )
                evac(R[:, j * 512:(j + 1) * 512], pc[:, :], bin_sb[:, :])

            # ---- butterfly levels 1..7
            for lv in range(1, 8):
                P = 2 ** (lv - 1)            # parent boxes at level lv-1
                Np = 2 ** (14 - lv)          # parent block columns
                Ncb = Np // 2                # child block columns
                S = min(512, Ncb)            # psum chunk columns
                TU = S // 64                 # t-pair units per chunk
                Rn = rp.tile([128, 8192], BF16, tag="R")
                for p in range(P):
                    pv = Rp_view = R[:, p * Np:(p + 1) * Np].rearrange(
                        "a (t2 two b) -> a t2 two b", two=2, b=64)
                    for cl in range(2):
                        box = 2 * p + cl
                        lhsT = wt[lv][:, box * 64:(box + 1) * 64]
                        for j in range(Ncb // S):
                            pc = ps.tile([128, 512], F32, tag="ps")
                            for q in range(2):
                                rhs = pv[:, j * TU:(j + 1) * TU, q, :]
                                nc.tensor.matmul(
                                    pc[64 * q:64 * (q + 1), 0:S], lhsT=lhsT, rhs=rhs,
                                    start=True, stop=True, tile_position=(0, 64 * q),
                                )
                            bc = 2 ** lv - 2 + box
                            evac(
                                Rn[:, box * Ncb + j * S: box * Ncb + (j + 1) * S],
                                pc[:, 0:S], bd7_sb[:, bc:bc + 1],
                            )
                R = Rn

            # ---- level 8: pair-packed, no parity split (V8 group tiles)
            Rn = rp.tile([128, 8192], BF16, tag="R")
            for p in range(128):
                pc = ps.tile([128, 512], F32, tag="ps")
                lhsT = w8_sb[p // 64][:, (p % 64) * 128:(p % 64) * 128 + 128]
                nc.tensor.matmul(
                    pc[:, 0:64], lhsT=lhsT, rhs=R[:, p * 64:(p + 1) * 64],
                    start=True, stop=True,
                )
                evac(Rn[:, p * 64:(p + 1) * 64], pc[:, 0:64], bp8_sb[:, p:p + 1])
            R = Rn

            # ---- output: y[b, 32p+m] = V8g[:, b] . Fea_pk[:, (p, m)]
            for bt in range(8):
                pco = po.tile([BC, 512], F32, tag="po")
                for g in range(16):
                    p = bt * 16 + g
                    nc.tensor.matmul(
                        pco[:, g * 32:(g + 1) * 32],
                        lhsT=R[:, p * 64:(p + 1) * 64],
                        rhs=fea_sb[:, p * 32:(p + 1) * 32],
                        start=True, stop=True,
                    )
                o_sb = op.tile([BC, 512], F32, tag="os")
                if bt % 2 == 0:
                    nc.scalar.copy(o_sb[:, :], pco[:, :])
                else:
                    nc.vector.tensor_copy(o_sb[:, :], pco[:, :])
                nc.sync.dma_start(out.ap()[:, bt * 512:(bt + 1) * 512], o_sb[:, :])

    nc.compile()
    return nc


def _make_runner(nc):
    """Cached jitted SPMD runner over the 8 cores (mirrors
    bass2jax.run_bass_via_pjrt, but reusable across calls and without
    donation — this kernel writes every output element)."""
    import jax

    from concourse.bass2jax import (
        _bass_exec_p,
        install_neuronx_cc_hook,
        partition_id_tensor,
    )
    from jax.experimental.shard_map import shard_map
    from jax.sharding import Mesh, PartitionSpec

    install_neuronx_cc_hook()

    partition_name = nc.partition_id_tensor.name if nc.partition_id_tensor else None
    in_names: list[str] = []
    out_names: list[str] = []
    out_avals = []
    zero_outs: list[np.ndarray] = []
    for alloc in nc.m.functions[0].allocations:
        if not isinstance(alloc, mybir.MemoryLocationSet):
            continue
        name = alloc.memorylocations[0].name
        if alloc.kind == "ExternalInput":
            if name != partition_name:
                in_names.append(name)
        elif alloc.kind == "ExternalOutput":
            shape = tuple(alloc.tensor_shape)
            dtype = mybir.dt.np(alloc.dtype)
            out_names.append(name)
            out_avals.append(jax.core.ShapedArray(shape, dtype))
            zero_outs.append(np.zeros(shape, dtype))
    n_params = len(in_names)
    all_names = in_names + out_names
    if partition_name is not None:
        all_names = all_names + [partition_name]

    def _body(*args):
        operands = list(args)
        if partition_name is not None:
            operands.append(partition_id_tensor())
        outs = _bass_exec_p.bind(
            *operands,
            out_avals=tuple(out_avals),
            in_names=tuple(all_names),
            out_names=tuple(out_names),
            lowering_input_output_aliases=(),
            sim_require_finite=True,
            sim_require_nnan=True,
            nc=nc,
        )
        return tuple(outs)

    devices = jax.devices()[:NCORES]
    mesh = Mesh(np.asarray(devices), ("core",))
    n_all = n_params + len(out_names)
    sharded = jax.jit(
        shard_map(
            _body, mesh=mesh,
            in_specs=(PartitionSpec("core"),) * n_all,
            out_specs=(PartitionSpec("core"),) * len(out_names),
            check_rep=False,
        ),
        keep_unused=True,
    )
    return {
        "fn": sharded,
        "in_names": in_names,
        "out_names": out_names,
        "out_avals": out_avals,
        "zero_outs": zero_outs,
    }


def _runner():
    if "nc" not in _CACHE:
        _CACHE["nc"] = _build_module()
    if "runner" not in _CACHE:
        _CACHE["runner"] = _make_runner(_CACHE["nc"])
    return _CACHE["runner"]


def _concat_args(in_maps):
    r = _runner()
    args = [
        np.concatenate([np.asarray(m[name]) for m in in_maps], axis=0)
        for name in r["in_names"]
    ]
    args += [
        np.zeros((NCORES * z.shape[0], *z.shape[1:]), z.dtype) for z in r["zero_outs"]
    ]
    return args


def kernel(**inputs) -> np.ndarray:
    r = _runner()
    shared = pack_shared(
        inputs["W_in"], inputs["b_in"], inputs["W_lvl"], inputs["b_lvl"], inputs["Fea"]
    )
    in_data = np.asarray(inputs["in_data"], np.float32)
    in_maps = []
    for c in range(NCORES):
        m = dict(shared)
        m["xt"] = pack_x(in_data[c * BC:(c + 1) * BC])
        in_maps.append(m)

    out_arrs = r["fn"](*_concat_args(in_maps))
    out = np.asarray(out_arrs[r["out_names"].index("out")])
    return out.reshape(B, KTOT * FOUT, 1).astype(np.float32)


# revision 31
# speedup vs baseline: 91.6990x; 1.1442x over previous
"""Trainium2 Bass kernel for the 8-level butterfly layer.

Contract: kernel(**inputs) takes FULL unsharded numpy inputs
(in_data [512,4096], W_in [16,64], b_in [64], W_lvl [510,2,64,64],
b_lvl [510,64], Fea [256,64,16]) and returns the FULL output
(512, 4096, 1) float32.

Strategy: pure data parallelism over batch (64 rows per core, 8 cores),
butterfly filters replicated. Per core, each level is a set of K=128
contraction matmuls in bf16. The t-parity split needed by the next
level's pair concatenation is produced by the matmul itself via
column-tiled PE matmuls (even-t columns -> PSUM partitions 0:64, odd-t
-> 64:128), so every PSUM->SBUF relu+bias+cast runs on all 128
partitions with no partition shifts.

Activation layout per level L ("pair format"), one SBUF tensor
R_L [128, 8192] bf16: box c of level L occupies columns
[c*N, (c+1)*N), N = 2^(13-L); partition (s*64 + ch) holds channel ch of
position t with parity s; column within the box block is (t//2)*64 + b.
"""

import numpy as np
import ml_dtypes

import concourse.bass as bass
import concourse.mybir as mybir
import concourse.tile as tile
from concourse import bacc

NCORES = 8
B = 512
BC = B // NCORES  # 64 batch rows per core
NLVL = 8
C = 64
FIN = 16
FOUT = 16
KTOT = 256  # 2**NLVL
INS = 4096

BF16 = mybir.dt.bfloat16
F32 = mybir.dt.float32

_CACHE: dict = {}
_PHASES: list = []  # (phase_name, next_instruction_id_at_start) from last build


def _bf16(a: np.ndarray) -> np.ndarray:
    return np.ascontiguousarray(a.astype(np.float32)).astype(ml_dtypes.bfloat16)


def pack_shared(W_in, b_in, W_lvl, b_lvl, Fea) -> dict:
    """Host-side repacking of the replicated filter tensors."""
    W_in = np.asarray(W_in, np.float32)
    b_in = np.asarray(b_in, np.float32)
    W_lvl = np.asarray(W_lvl, np.float32)
    b_lvl = np.asarray(b_lvl, np.float32)
    Fea = np.asarray(Fea, np.float32)

    # Levels 1..7 weights: boxes are W_lvl[0:254] in level-major order.
    # [254, 2, 64, 64] -> [254, 128, 64] (row = s*64+c_in) -> [128, 254*64]
    wmain = W_lvl[0:254].reshape(254, 128, 64).transpose(1, 0, 2).reshape(128, 254 * 64)

    # Level 8 weights, pair-packed: pair p holds boxes 2p, 2p+1
    # (global idx 254+2p, 254+2p+1). [128, (p, j, m)] -> [128, 16384]
    w8 = W_lvl[254:510].reshape(128, 2, 128, 64).transpose(2, 0, 1, 3).reshape(128, 128 * 128)

    # Fea pair-packed blockdiag: [128 rows (j,ch), 128 pairs, 32]
    fea = np.zeros((128, 128, 32), np.float32)
    fea[0:64, :, 0:16] = Fea[0::2].transpose(1, 0, 2)
    fea[64:128, :, 16:32] = Fea[1::2].transpose(1, 0, 2)
    fea = fea.reshape(128, 128 * 32)

    # Biases (fp32): duplicated across partition halves for levels in..5.
    bin_h = np.concatenate([b_in, b_in]).reshape(128, 1)
    bd7 = np.concatenate([b_lvl[0:254], b_lvl[0:254]], axis=1).T.copy()  # [128, 254]

    # Input filter, K-stacked blockdiag: rows 0:16 feed even-k channels
    # (psum partitions 0:64), rows 16:32 odd-k (partitions 64:128).
    winp = np.zeros((32, 128), np.float32)
    winp[0:16, 0:64] = W_in
    winp[16:32, 64:128] = W_in

    # Levels 6-8 bias-matmul operands: psum[p, col] += lhsT[blk(col), p].
    # l6: 16 psums x 4 boxes x 128 cols; l7: 16 x 8 x 64; l8: 16 x 8 parents.
    b6 = b_lvl[62:126]          # level-6 boxes, [64, 64]
    bias6 = np.concatenate([b6, b6], axis=1).reshape(16, 4, 128)
    bias6 = bias6.transpose(1, 0, 2).reshape(4, 2048)
    b7 = b_lvl[126:254]
    bias7 = np.concatenate([b7, b7], axis=1).reshape(16, 8, 128)
    bias7 = bias7.transpose(1, 0, 2).reshape(8, 2048)
    b8 = b_lvl[254:510].reshape(128, 128)  # pair p rows (j,ch)
    bias8 = b8.reshape(16, 8, 128).transpose(1, 0, 2).reshape(8, 2048)
    ones4 = np.repeat(np.eye(4, dtype=np.float32), 128, axis=1)   # [4, 512]
    ones8 = np.repeat(np.eye(8, dtype=np.float32), 64, axis=1)    # [8, 512]

    return {
        "winp": _bf16(winp),
        "wmain": _bf16(wmain),
        "w8": _bf16(w8),
        "fea": _bf16(fea),
        "bin": np.ascontiguousarray(bin_h, np.float32),
        "bd7": np.ascontiguousarray(bd7, np.float32),
        "bias6": _bf16(bias6),
        "bias7": _bf16(bias7),
        "bias8": _bf16(bias8),
        "ones4": _bf16(ones4),
        "ones8": _bf16(ones8),
    }


def pack_x(x_shard: np.ndarray) -> np.ndarray:
    """[64, 4096] batch shard -> [32, 8192] bf16: row h*16+f holds
    x[b, (2k'+h)*16+f] at col k'*64+b (K-stacked even/odd k)."""
    xs = np.asarray(x_shard, np.float32).reshape(BC, 128, 2, FIN)
    return _bf16(xs.transpose(2, 3, 1, 0).reshape(32, 128 * BC))


def _build_module(loop_iters: int | None = None):
    """Build the bass module. loop_iters wraps the whole body in an
    on-device For_i loop (benchmarking only; graded path uses None)."""
    nc = bacc.Bacc("TRN2", target_bir_lowering=False, debug=False)

    xt = nc.dram_tensor("xt", [32, 128 * BC], BF16, kind="ExternalInput")
    winp = nc.dram_tensor("winp", [32, 128], BF16, kind="ExternalInput")
    wmain = nc.dram_tensor("wmain", [128, 254 * 64], BF16, kind="ExternalInput")
    w8 = nc.dram_tensor("w8", [128, 128 * 128], BF16, kind="ExternalInput")
    fea = nc.dram_tensor("fea", [128, 128 * 32], BF16, kind="ExternalInput")
    bin_t = nc.dram_tensor("bin", [128, 1], F32, kind="ExternalInput")
    bd7 = nc.dram_tensor("bd7", [128, 254], F32, kind="ExternalInput")
    bias6 = nc.dram_tensor("bias6", [4, 2048], BF16, kind="ExternalInput")
    bias7 = nc.dram_tensor("bias7", [8, 2048], BF16, kind="ExternalInput")
    bias8 = nc.dram_tensor("bias8", [8, 2048], BF16, kind="ExternalInput")
    ones4 = nc.dram_tensor("ones4", [4, 512], BF16, kind="ExternalInput")
    ones8 = nc.dram_tensor("ones8", [8, 512], BF16, kind="ExternalInput")
    out = nc.dram_tensor("out", [BC, KTOT * FOUT], F32, kind="ExternalOutput")

    relu = mybir.ActivationFunctionType.Relu
    evac_cnt = 0
    _PHASES.clear()

    def mark(name):
        _PHASES.append((name, int(nc.get_next_instruction_name().split("-")[1])))

    def evac(dst, src, bias_ap=None):
        nonlocal evac_cnt
        if evac_cnt % 2 == 0:
            nc.scalar.activation(dst, src, relu,
                                 bias=bias_ap if bias_ap is not None else 0.0)
        elif bias_ap is not None:
            nc.vector.tensor_scalar(
                dst, src, bias_ap, 0.0,
                op0=mybir.AluOpType.add, op1=mybir.AluOpType.max,
            )
        else:
            nc.vector.tensor_scalar(
                dst, src, 0.0, None, op0=mybir.AluOpType.max,
            )
        evac_cnt += 1

    import contextlib

    with tile.TileContext(nc) as tc:
        with (
            tc.tile_pool(name="wp", bufs=1) as wp,
            tc.tile_pool(name="xp", bufs=1) as xp,
            tc.tile_pool(name="rp", bufs=3) as rp,
            tc.tile_pool(name="op", bufs=3) as op,
            tc.tile_pool(name="ps", bufs=4, space="PSUM") as ps,
            tc.tile_pool(name="po", bufs=3, space="PSUM") as po,
            tc.For_i(0, loop_iters, 1) if loop_iters else contextlib.nullcontext(),
        ):
            x_sb = xp.tile([32, 128 * BC], BF16, tag="x")
            nc.sync.dma_start(x_sb[:, :], xt.ap())
            winp_sb = wp.tile([32, 128], BF16, tag="winp")
            nc.sync.dma_start(winp_sb[:, :], winp.ap())
            bin_sb = wp.tile([128, 1], F32, tag="bin")
            nc.sync.dma_start(bin_sb[:, :], bin_t.ap())
            bd7_sb = wp.tile([128, 254], F32, tag="bd7")
            nc.sync.dma_start(bd7_sb[:, :], bd7.ap())
            bias_sb = {}
            for nm, t, kdim in (("bias6", bias6, 4), ("bias7", bias7, 8),
                                ("bias8", bias8, 8)):
                bias_sb[nm] = wp.tile([kdim, 2048], BF16, tag=nm, name=nm)
                nc.sync.dma_start(bias_sb[nm][:, :], t.ap())
            ones4_sb = wp.tile([4, 512], BF16, tag="ones4")
            nc.sync.dma_start(ones4_sb[:, :], ones4.ap())
            ones8_sb = wp.tile([8, 512], BF16, tag="ones8")
            nc.sync.dma_start(ones8_sb[:, :], ones8.ap())

            wt = {}
            for lv in range(1, 8):
                nb = 2 ** lv  # boxes at this level
                wt[lv] = wp.tile([128, nb * 64], BF16, tag=f"wt{lv}", name=f"wt{lv}")
                c0 = (2 ** lv - 2) * 64
                nc.sync.dma_start(wt[lv][:, :], wmain.ap()[:, c0:c0 + nb * 64])
            w8_sb = []
            for h in range(2):
                t = wp.tile([128, 8192], BF16, tag=f"w8{h}", name=f"w8{h}")
                nc.sync.dma_start(t[:, :], w8.ap()[:, h * 8192:(h + 1) * 8192])
                w8_sb.append(t)
            fea_sb = wp.tile([128, 128 * 32], BF16, tag="fea")
            nc.sync.dma_start(fea_sb[:, :], fea.ap())

            # ---- input interpolation: K-stacked [32,128] blockdiag W_in
            mark("input")
            R = rp.tile([128, 8192], BF16, tag="R")
            for j in range(16):
                pc = ps.tile([128, 512], F32, tag="ps")
                nc.tensor.matmul(
                    pc[:, :], lhsT=winp_sb[:, :],
                    rhs=x_sb[:, j * 512:(j + 1) * 512],
                    start=True, stop=True,
                )
                evac(R[:, j * 512:(j + 1) * 512], pc[:, :], bin_sb[:, :])

            # ---- butterfly levels 1..5 (per-box psum chunks, bias in evac)
            for lv in range(1, 6):
                mark(f"l{lv}")
                P = 2 ** (lv - 1)            # parent boxes at level lv-1
                Np = 2 ** (14 - lv)          # parent block columns
                Ncb = Np // 2                # child block columns
                S = min(512, Ncb)            # psum chunk columns
                TU = S // 64                 # t-pair units per chunk
                Rn = rp.tile([128, 8192], BF16, tag="R")
                for p in range(P):
                    pv = Rp_view = R[:, p * Np:(p + 1) * Np].rearrange(
                        "a (t2 two b) -> a t2 two b", two=2, b=64)
                    for cl in range(2):
                        box = 2 * p + cl
                        lhsT = wt[lv][:, box * 64:(box + 1) * 64]
                        for j in range(Ncb // S):
                            pc = ps.tile([128, 512], F32, tag="ps")
                            for q in range(2):
                                rhs = pv[:, j * TU:(j + 1) * TU, q, :]
                                nc.tensor.matmul(
                                    pc[64 * q:64 * (q + 1), 0:S], lhsT=lhsT, rhs=rhs,
                                    start=True, stop=True, tile_position=(0, 64 * q),
                                )
                            bc = 2 ** lv - 2 + box
                            evac(
                                Rn[:, box * Ncb + j * S: box * Ncb + (j + 1) * S],
                                pc[:, 0:S], bd7_sb[:, bc:bc + 1],
                            )
                R = Rn

            # ---- levels 6-7: bias via K=4/8 ones-pattern matmul, merged
            # 512-col psums + single relu-only evac per psum.
            for lv, nbx, ones_sb in ((6, 4, ones4_sb), (7, 8, ones8_sb)):
                mark(f"l{lv}")
                Np = 2 ** (14 - lv)
                Ncb = Np // 2
                bsb = bias_sb[f"bias{lv}"]
                Rn = rp.tile([128, 8192], BF16, tag="R")
                for i in range(16):
                    pc = ps.tile([128, 512], F32, tag="ps")
                    nc.tensor.matmul(
                        pc[:, :], lhsT=bsb[:, i * 128:(i + 1) * 128],
                        rhs=ones_sb[:, :], start=True, stop=False,
                    )
                    for bl in range(nbx):
                        box = nbx * i + bl
                        p = box // 2
                        pv = R[:, p * Np:(p + 1) * Np].rearrange(
                            "a (t2 two b) -> a t2 two b", two=2, b=64)
                        lhsT = wt[lv][:, box * 64:(box + 1) * 64]
                        for q in range(2):
                            nc.tensor.matmul(
                                pc[64 * q:64 * (q + 1),
                                   bl * Ncb:(bl + 1) * Ncb],
                                lhsT=lhsT, rhs=pv[:, :, q, :],
                                start=False, stop=True,
                                tile_position=(0, 64 * q),
                            )
                    evac(Rn[:, i * 512:(i + 1) * 512], pc[:, :])
                R = Rn

            # ---- level 8 (pair-packed, bias matmul, merged psums) with the
            # output stage interleaved: out batch bt consumes V8 groups
            # 16bt..16bt+15 = l8 psums i=2bt, 2bt+1.
            mark("l8")
            Rn = rp.tile([128, 8192], BF16, tag="R")
            for i in range(16):
                pc = ps.tile([128, 512], F32, tag="ps")
                nc.tensor.matmul(
                    pc[:, :], lhsT=bias_sb["bias8"][:, i * 128:(i + 1) * 128],
                    rhs=ones8_sb[:, :], start=True, stop=False,
                )
                for pl in range(8):
                    p = 8 * i + pl
                    lhsT = w8_sb[p // 64][:, (p % 64) * 128:(p % 64) * 128 + 128]
                    nc.tensor.matmul(
                        pc[:, pl * 64:(pl + 1) * 64], lhsT=lhsT,
                        rhs=R[:, p * 64:(p + 1) * 64],
                        start=False, stop=True,
                    )
                evac(Rn[:, i * 512:(i + 1) * 512], pc[:, :])
                if i % 2 == 1:
                    # out batch for V8 groups of psums i-1, i
                    bt = i // 2
                    pco = po.tile([BC, 512], F32, tag="po")
                    for g in range(16):
                        p = bt * 16 + g
                        nc.tensor.matmul(
                            pco[:, g * 32:(g + 1) * 32],
                            lhsT=Rn[:, p * 64:(p + 1) * 64],
                            rhs=fea_sb[:, p * 32:(p + 1) * 32],
                            start=True, stop=True,
                        )
                    o_sb = op.tile([BC, 512], F32, tag="os")
                    if bt % 2 == 0:
                        nc.scalar.copy(o_sb[:, :], pco[:, :])
                    else:
                        nc.vector.tensor_copy(o_sb[:, :], pco[:, :])
                    nc.sync.dma_start(out.ap()[:, bt * 512:(bt + 1) * 512],
                                      o_sb[:, :])
            R = Rn

    nc.compile()
    return nc


def _make_runner(nc):
    """Cached jitted SPMD runner over the 8 cores (mirrors
    bass2jax.run_bass_via_pjrt, but reusable across calls and without
    donation — this kernel writes every output element)."""
    import jax

    from concourse.bass2jax import (
        _bass_exec_p,
        install_neuronx_cc_hook,
        partition_id_tensor,
    )
    from jax.experimental.shard_map import shard_map
    from jax.sharding import Mesh, PartitionSpec

    install_neuronx_cc_hook()

    partition_name = nc.partition_id_tensor.name if nc.partition_id_tensor else None
    in_names: list[str] = []
    out_names: list[str] = []
    out_avals = []
    zero_outs: list[np.ndarray] = []
    for alloc in nc.m.functions[0].allocations:
        if not isinstance(alloc, mybir.MemoryLocationSet):
            continue
        name = alloc.memorylocations[0].name
        if alloc.kind == "ExternalInput":
            if name != partition_name:
                in_names.append(name)
        elif alloc.kind == "ExternalOutput":
            shape = tuple(alloc.tensor_shape)
            dtype = mybir.dt.np(alloc.dtype)
            out_names.append(name)
            out_avals.append(jax.core.ShapedArray(shape, dtype))
            zero_outs.append(np.zeros(shape, dtype))
    n_params = len(in_names)
    all_names = in_names + out_names
    if partition_name is not None:
        all_names = all_names + [partition_name]

    def _body(*args):
        operands = list(args)
        if partition_name is not None:
            operands.append(partition_id_tensor())
        outs = _bass_exec_p.bind(
            *operands,
            out_avals=tuple(out_avals),
            in_names=tuple(all_names),
            out_names=tuple(out_names),
            lowering_input_output_aliases=(),
            sim_require_finite=True,
            sim_require_nnan=True,
            nc=nc,
        )
        return tuple(outs)

    devices = jax.devices()[:NCORES]
    mesh = Mesh(np.asarray(devices), ("core",))
    n_all = n_params + len(out_names)
    sharded = jax.jit(
        shard_map(
            _body, mesh=mesh,
            in_specs=(PartitionSpec("core"),) * n_all,
            out_specs=(PartitionSpec("core"),) * len(out_names),
            check_rep=False,
        ),
        keep_unused=True,
    )
    return {
        "fn": sharded,
        "in_names": in_names,
        "out_names": out_names,
        "out_avals": out_avals,
        "zero_outs": zero_outs,
    }


def _runner():
    if "nc" not in _CACHE:
        _CACHE["nc"] = _build_module()
    if "runner" not in _CACHE:
        _CACHE["runner"] = _make_runner(_CACHE["nc"])
    return _CACHE["runner"]


def _concat_args(in_maps):
    r = _runner()
    args = [
        np.concatenate([np.asarray(m[name]) for m in in_maps], axis=0)
        for name in r["in_names"]
    ]
    args += [
        np.zeros((NCORES * z.shape[0], *z.shape[1:]), z.dtype) for z in r["zero_outs"]
    ]
    return args


def kernel(**inputs) -> np.ndarray:
    r = _runner()
    shared = pack_shared(
        inputs["W_in"], inputs["b_in"], inputs["W_lvl"], inputs["b_lvl"], inputs["Fea"]
    )
    in_data = np.asarray(inputs["in_data"], np.float32)
    in_maps = []
    for c in range(NCORES):
        m = dict(shared)
        m["xt"] = pack_x(in_data[c * BC:(c + 1) * BC])
        in_maps.append(m)

    out_arrs = r["fn"](*_concat_args(in_maps))
    out = np.asarray(out_arrs[r["out_names"].index("out")])
    return out.reshape(B, KTOT * FOUT, 1).astype(np.float32)


# revision 36
# speedup vs baseline: 92.5293x; 1.0091x over previous
"""Trainium2 Bass kernel for the 8-level butterfly layer.

Contract: kernel(**inputs) takes FULL unsharded numpy inputs
(in_data [512,4096], W_in [16,64], b_in [64], W_lvl [510,2,64,64],
b_lvl [510,64], Fea [256,64,16]) and returns the FULL output
(512, 4096, 1) float32.

Strategy: pure data parallelism over batch (64 rows per core, 8 cores),
butterfly filters replicated. Per core, each level is a set of K=128
contraction matmuls in bf16. The t-parity split needed by the next
level's pair concatenation is produced by the matmul itself via
column-tiled PE matmuls (even-t columns -> PSUM partitions 0:64, odd-t
-> 64:128), so every PSUM->SBUF relu+bias+cast runs on all 128
partitions with no partition shifts.

Activation layout per level L ("pair format"), one SBUF tensor
R_L [128, 8192] bf16: box c of level L occupies columns
[c*N, (c+1)*N), N = 2^(13-L); partition (s*64 + ch) holds channel ch of
position t with parity s; column within the box block is (t//2)*64 + b.
"""

import numpy as np
import ml_dtypes

import concourse.bass as bass
import concourse.mybir as mybir
import concourse.tile as tile
from concourse import bacc

NCORES = 8
B = 512
BC = B // NCORES  # 64 batch rows per core
NLVL = 8
C = 64
FIN = 16
FOUT = 16
KTOT = 256  # 2**NLVL
INS = 4096

BF16 = mybir.dt.bfloat16
F32 = mybir.dt.float32

_CACHE: dict = {}
_PHASES: list = []  # (phase_name, next_instruction_id_at_start) from last build


def _bf16(a: np.ndarray) -> np.ndarray:
    return np.ascontiguousarray(a.astype(np.float32)).astype(ml_dtypes.bfloat16)


def pack_shared(W_in, b_in, W_lvl, b_lvl, Fea) -> dict:
    """Host-side repacking of the replicated filter tensors."""
    W_in = np.asarray(W_in, np.float32)
    b_in = np.asarray(b_in, np.float32)
    W_lvl = np.asarray(W_lvl, np.float32)
    b_lvl = np.asarray(b_lvl, np.float32)
    Fea = np.asarray(Fea, np.float32)

    # Levels 1..7 weights: boxes are W_lvl[0:254] in level-major order.
    # [254, 2, 64, 64] -> [254, 128, 64] (row = s*64+c_in) -> [128, 254*64]
    wmain = W_lvl[0:254].reshape(254, 128, 64).transpose(1, 0, 2).reshape(128, 254 * 64)

    # Level 8 weights, pair-packed: pair p holds boxes 2p, 2p+1
    # (global idx 254+2p, 254+2p+1). [128, (p, j, m)] -> [128, 16384]
    w8 = W_lvl[254:510].reshape(128, 2, 128, 64).transpose(2, 0, 1, 3).reshape(128, 128 * 128)

    # Fea pair-packed blockdiag: [128 rows (j,ch), 128 pairs, 32]
    fea = np.zeros((128, 128, 32), np.float32)
    fea[0:64, :, 0:16] = Fea[0::2].transpose(1, 0, 2)
    fea[64:128, :, 16:32] = Fea[1::2].transpose(1, 0, 2)
    fea = fea.reshape(128, 128 * 32)

    # Biases (fp32): duplicated across partition halves for levels in..5.
    bin_h = np.concatenate([b_in, b_in]).reshape(128, 1)
    bd7 = np.concatenate([b_lvl[0:254], b_lvl[0:254]], axis=1).T.copy()  # [128, 254]

    # Input filter, K-stacked blockdiag: rows 0:16 feed even-k channels
    # (psum partitions 0:64), rows 16:32 odd-k (partitions 64:128).
    winp = np.zeros((32, 128), np.float32)
    winp[0:16, 0:64] = W_in
    winp[16:32, 64:128] = W_in

    # Levels 6-8 bias-matmul operands: psum[p, col] += lhsT[blk(col), p].
    # l6: 16 psums x 4 boxes x 128 cols; l7: 16 x 8 x 64; l8: 16 x 8 parents.
    b6 = b_lvl[62:126]          # level-6 boxes, [64, 64]
    bias6 = np.concatenate([b6, b6], axis=1).reshape(16, 4, 128)
    bias6 = bias6.transpose(1, 0, 2).reshape(4, 2048)
    b7 = b_lvl[126:254]
    bias7 = np.concatenate([b7, b7], axis=1).reshape(16, 8, 128)
    bias7 = bias7.transpose(1, 0, 2).reshape(8, 2048)
    b8 = b_lvl[254:510].reshape(128, 128)  # pair p rows (j,ch)
    bias8 = b8.reshape(16, 8, 128).transpose(1, 0, 2).reshape(8, 2048)
    ones4 = np.repeat(np.eye(4, dtype=np.float32), 128, axis=1)   # [4, 512]
    ones8 = np.repeat(np.eye(8, dtype=np.float32), 64, axis=1)    # [8, 512]

    return {
        "winp": _bf16(winp),
        "wmain": _bf16(wmain),
        "w8": _bf16(w8),
        "fea": _bf16(fea),
        "bin": np.ascontiguousarray(bin_h, np.float32),
        "bd7": np.ascontiguousarray(bd7, np.float32),
        "bias6": _bf16(bias6),
        "bias7": _bf16(bias7),
        "bias8": _bf16(bias8),
        "ones4": _bf16(ones4),
        "ones8": _bf16(ones8),
    }


def pack_x(x_shard: np.ndarray) -> np.ndarray:
    """[64, 4096] batch shard -> [32, 8192] bf16: row h*16+f holds
    x[b, (2k'+h)*16+f] at col k'*64+b (K-stacked even/odd k)."""
    xs = np.asarray(x_shard, np.float32).reshape(BC, 128, 2, FIN)
    return _bf16(xs.transpose(2, 3, 1, 0).reshape(32, 128 * BC))


def _build_module(loop_iters: int | None = None):
    """Build the bass module. loop_iters wraps the whole body in an
    on-device For_i loop (benchmarking only; graded path uses None)."""
    nc = bacc.Bacc("TRN2", target_bir_lowering=False, debug=False)

    xt = nc.dram_tensor("xt", [32, 128 * BC], BF16, kind="ExternalInput")
    winp = nc.dram_tensor("winp", [32, 128], BF16, kind="ExternalInput")
    wmain = nc.dram_tensor("wmain", [128, 254 * 64], BF16, kind="ExternalInput")
    w8 = nc.dram_tensor("w8", [128, 128 * 128], BF16, kind="ExternalInput")
    fea = nc.dram_tensor("fea", [128, 128 * 32], BF16, kind="ExternalInput")
    bin_t = nc.dram_tensor("bin", [128, 1], F32, kind="ExternalInput")
    bd7 = nc.dram_tensor("bd7", [128, 254], F32, kind="ExternalInput")
    bias6 = nc.dram_tensor("bias6", [4, 2048], BF16, kind="ExternalInput")
    bias7 = nc.dram_tensor("bias7", [8, 2048], BF16, kind="ExternalInput")
    bias8 = nc.dram_tensor("bias8", [8, 2048], BF16, kind="ExternalInput")
    ones4 = nc.dram_tensor("ones4", [4, 512], BF16, kind="ExternalInput")
    ones8 = nc.dram_tensor("ones8", [8, 512], BF16, kind="ExternalInput")
    out = nc.dram_tensor("out", [BC, KTOT * FOUT], F32, kind="ExternalOutput")

    relu = mybir.ActivationFunctionType.Relu
    evac_cnt = 0
    _PHASES.clear()

    def mark(name):
        _PHASES.append((name, int(nc.get_next_instruction_name().split("-")[1])))

    def evac(dst, src, bias_ap=None):
        nonlocal evac_cnt
        if evac_cnt % 2 == 0:
            nc.scalar.activation(dst, src, relu,
                                 bias=bias_ap if bias_ap is not None else 0.0)
        elif bias_ap is not None:
            nc.vector.tensor_scalar(
                dst, src, bias_ap, 0.0,
                op0=mybir.AluOpType.add, op1=mybir.AluOpType.max,
            )
        else:
            nc.vector.tensor_scalar(
                dst, src, 0.0, None, op0=mybir.AluOpType.max,
            )
        evac_cnt += 1

    import contextlib

    with tile.TileContext(nc) as tc:
        with (
            tc.tile_pool(name="wp", bufs=1) as wp,
            tc.tile_pool(name="xp", bufs=1) as xp,
            tc.tile_pool(name="rp", bufs=3) as rp,
            tc.tile_pool(name="op", bufs=3) as op,
            tc.tile_pool(name="ps", bufs=5, space="PSUM") as ps,
            tc.tile_pool(name="po", bufs=3, space="PSUM") as po,
            tc.For_i(0, loop_iters, 1) if loop_iters else contextlib.nullcontext(),
        ):
            x_sb = xp.tile([32, 128 * BC], BF16, tag="x")
            nc.sync.dma_start(x_sb[:, :], xt.ap())
            winp_sb = wp.tile([32, 128], BF16, tag="winp")
            nc.sync.dma_start(winp_sb[:, :], winp.ap())
            bin_sb = wp.tile([128, 1], F32, tag="bin")
            nc.sync.dma_start(bin_sb[:, :], bin_t.ap())
            bd7_sb = wp.tile([128, 254], F32, tag="bd7")
            nc.sync.dma_start(bd7_sb[:, :], bd7.ap())
            bias_sb = {}
            for nm, t, kdim in (("bias6", bias6, 4), ("bias7", bias7, 8),
                                ("bias8", bias8, 8)):
                bias_sb[nm] = wp.tile([kdim, 2048], BF16, tag=nm, name=nm)
                nc.sync.dma_start(bias_sb[nm][:, :], t.ap())
            ones4_sb = wp.tile([4, 512], BF16, tag="ones4")
            nc.sync.dma_start(ones4_sb[:, :], ones4.ap())
            ones8_sb = wp.tile([8, 512], BF16, tag="ones8")
            nc.sync.dma_start(ones8_sb[:, :], ones8.ap())

            wt = {}
            for lv in range(1, 8):
                nb = 2 ** lv  # boxes at this level
                wt[lv] = wp.tile([128, nb * 64], BF16, tag=f"wt{lv}", name=f"wt{lv}")
                c0 = (2 ** lv - 2) * 64
                nc.sync.dma_start(wt[lv][:, :], wmain.ap()[:, c0:c0 + nb * 64])
            w8_sb = []
            for h in range(2):
                t = wp.tile([128, 8192], BF16, tag=f"w8{h}", name=f"w8{h}")
                nc.sync.dma_start(t[:, :], w8.ap()[:, h * 8192:(h + 1) * 8192])
                w8_sb.append(t)
            fea_sb = wp.tile([128, 128 * 32], BF16, tag="fea")
            nc.sync.dma_start(fea_sb[:, :], fea.ap())

            # ---- input interpolation: K-stacked [32,128] blockdiag W_in
            mark("input")
            R = rp.tile([128, 8192], BF16, tag="R")
            for j in range(16):
                pc = ps.tile([128, 512], F32, tag="ps")
                nc.tensor.matmul(
                    pc[:, :], lhsT=winp_sb[:, :],
                    rhs=x_sb[:, j * 512:(j + 1) * 512],
                    start=True, stop=True,
                )
                evac(R[:, j * 512:(j + 1) * 512], pc[:, :], bin_sb[:, :])

            # ---- butterfly levels 1..5 (per-box psum chunks, bias in evac)
            for lv in range(1, 6):
                mark(f"l{lv}")
                P = 2 ** (lv - 1)            # parent boxes at level lv-1
                Np = 2 ** (14 - lv)          # parent block columns
                Ncb = Np // 2                # child block columns
                S = min(512, Ncb)            # psum chunk columns
                TU = S // 64                 # t-pair units per chunk
                Rn = rp.tile([128, 8192], BF16, tag="R")
                for p in range(P):
                    pv = Rp_view = R[:, p * Np:(p + 1) * Np].rearrange(
                        "a (t2 two b) -> a t2 two b", two=2, b=64)
                    for cl in range(2):
                        box = 2 * p + cl
                        lhsT = wt[lv][:, box * 64:(box + 1) * 64]
                        for j in range(Ncb // S):
                            pc = ps.tile([128, 512], F32, tag="ps")
                            for q in range(2):
                                rhs = pv[:, j * TU:(j + 1) * TU, q, :]
                                nc.tensor.matmul(
                                    pc[64 * q:64 * (q + 1), 0:S], lhsT=lhsT, rhs=rhs,
                                    start=True, stop=True, tile_position=(0, 64 * q),
                                )
                            bc = 2 ** lv - 2 + box
                            evac(
                                Rn[:, box * Ncb + j * S: box * Ncb + (j + 1) * S],
                                pc[:, 0:S], bd7_sb[:, bc:bc + 1],
                            )
                R = Rn

            # ---- levels 6-7: bias via K=4/8 ones-pattern matmul, merged
            # 512-col psums + single relu-only evac per psum.
            for lv, nbx, ones_sb in ((6, 4, ones4_sb), (7, 8, ones8_sb)):
                mark(f"l{lv}")
                Np = 2 ** (14 - lv)
                Ncb = Np // 2
                bsb = bias_sb[f"bias{lv}"]
                Rn = rp.tile([128, 8192], BF16, tag="R")
                for i in range(16):
                    pc = ps.tile([128, 512], F32, tag="ps")
                    nc.tensor.matmul(
                        pc[:, :], lhsT=bsb[:, i * 128:(i + 1) * 128],
                        rhs=ones_sb[:, :], start=True, stop=False,
                    )
                    for bl in range(nbx):
                        box = nbx * i + bl
                        p = box // 2
                        pv = R[:, p * Np:(p + 1) * Np].rearrange(
                            "a (t2 two b) -> a t2 two b", two=2, b=64)
                        lhsT = wt[lv][:, box * 64:(box + 1) * 64]
                        for q in range(2):
                            nc.tensor.matmul(
                                pc[64 * q:64 * (q + 1),
                                   bl * Ncb:(bl + 1) * Ncb],
                                lhsT=lhsT, rhs=pv[:, :, q, :],
                                start=False, stop=True,
                                tile_position=(0, 64 * q),
                            )
                    evac(Rn[:, i * 512:(i + 1) * 512], pc[:, :])
                R = Rn

            # ---- level 8 (pair-packed, bias matmul, merged psums) with the
            # output stage interleaved: out batch bt consumes V8 groups
            # 16bt..16bt+15 = l8 psums i=2bt, 2bt+1.
            mark("l8")
            Rn = rp.tile([128, 8192], BF16, tag="R")
            for i in range(16):
                pc = ps.tile([128, 512], F32, tag="ps")
                nc.tensor.matmul(
                    pc[:, :], lhsT=bias_sb["bias8"][:, i * 128:(i + 1) * 128],
                    rhs=ones8_sb[:, :], start=True, stop=False,
                )
                for pl in range(8):
                    p = 8 * i + pl
                    lhsT = w8_sb[p // 64][:, (p % 64) * 128:(p % 64) * 128 + 128]
                    nc.tensor.matmul(
                        pc[:, pl * 64:(pl + 1) * 64], lhsT=lhsT,
                        rhs=R[:, p * 64:(p + 1) * 64],
                        start=False, stop=True,
                    )
                evac(Rn[:, i * 512:(i + 1) * 512], pc[:, :])
                if i % 2 == 1:
                    # out batch for V8 groups of psums i-1, i
                    bt = i // 2
                    pco = po.tile([BC, 512], F32, tag="po")
                    for g in range(16):
                        p = bt * 16 + g
                        nc.tensor.matmul(
                            pco[:, g * 32:(g + 1) * 32],
                            lhsT=Rn[:, p * 64:(p + 1) * 64],
                            rhs=fea_sb[:, p * 32:(p + 1) * 32],
                            start=True, stop=True,
                        )
                    o_sb = op.tile([BC, 512], F32, tag="os")
                    if bt % 2 == 0:
                        nc.scalar.copy(o_sb[:, :], pco[:, :])
                    else:
                        nc.vector.tensor_copy(o_sb[:, :], pco[:, :])
                    nc.sync.dma_start(out.ap()[:, bt * 512:(bt + 1) * 512],
                                      o_sb[:, :])
            R = Rn

    nc.compile()
    return nc


def _make_runner(nc):
    """Cached jitted SPMD runner over the 8 cores (mirrors
    bass2jax.run_bass_via_pjrt, but reusable across calls and without
    donation — this kernel writes every output element)."""
    import jax

    from concourse.bass2jax import (
        _bass_exec_p,
        install_neuronx_cc_hook,
        partition_id_tensor,
    )
    from jax.experimental.shard_map import shard_map
    from jax.sharding import Mesh, PartitionSpec

    install_neuronx_cc_hook()

    partition_name = nc.partition_id_tensor.name if nc.partition_id_tensor else None
    in_names: list[str] = []
    out_names: list[str] = []
    out_avals = []
    zero_outs: list[np.ndarray] = []
    for alloc in nc.m.functions[0].allocations:
        if not isinstance(alloc, mybir.MemoryLocationSet):
            continue
        name = alloc.memorylocations[0].name
        if alloc.kind == "ExternalInput":
            if name != partition_name:
                in_names.append(name)
        elif alloc.kind == "ExternalOutput":
            shape = tuple(alloc.tensor_shape)
            dtype = mybir.dt.np(alloc.dtype)
            out_names.append(name)
            out_avals.append(jax.core.ShapedArray(shape, dtype))
            zero_outs.append(np.zeros(shape, dtype))
    n_params = len(in_names)
    all_names = in_names + out_names
    if partition_name is not None:
        all_names = all_names + [partition_name]

    def _body(*args):
        operands = list(args)
        if partition_name is not None:
            operands.append(partition_id_tensor())
        outs = _bass_exec_p.bind(
            *operands,
            out_avals=tuple(out_avals),
            in_names=tuple(all_names),
            out_names=tuple(out_names),
            lowering_input_output_aliases=(),
            sim_require_finite=True,
            sim_require_nnan=True,
            nc=nc,
        )
        return tuple(outs)

    devices = jax.devices()[:NCORES]
    mesh = Mesh(np.asarray(devices), ("core",))
    n_all = n_params + len(out_names)
    sharded = jax.jit(
        shard_map(
            _body, mesh=mesh,
            in_specs=(PartitionSpec("core"),) * n_all,
            out_specs=(PartitionSpec("core"),) * len(out_names),
            check_rep=False,
        ),
        keep_unused=True,
    )
    return {
        "fn": sharded,
        "in_names": in_names,
        "out_names": out_names,
        "out_avals": out_avals,
        "zero_outs": zero_outs,
    }


def _runner():
    if "nc" not in _CACHE:
        _CACHE["nc"] = _build_module()
    if "runner" not in _CACHE:
        _CACHE["runner"] = _make_runner(_CACHE["nc"])
    return _CACHE["runner"]


def _concat_args(in_maps):
    r = _runner()
    args = [
        np.concatenate([np.asarray(m[name]) for m in in_maps], axis=0)
        for name in r["in_names"]
    ]
    args += [
        np.zeros((NCORES * z.shape[0], *z.shape[1:]), z.dtype) for z in r["zero_outs"]
    ]
    return args


def kernel(**inputs) -> np.ndarray:
    r = _runner()
    shared = pack_shared(
        inputs["W_in"], inputs["b_in"], inputs["W_lvl"], inputs["b_lvl"], inputs["Fea"]
    )
    in_data = np.asarray(inputs["in_data"], np.float32)
    in_maps = []
    for c in range(NCORES):
        m = dict(shared)
        m["xt"] = pack_x(in_data[c * BC:(c + 1) * BC])
        in_maps.append(m)

    out_arrs = r["fn"](*_concat_args(in_maps))
    out = np.asarray(out_arrs[r["out_names"].index("out")])
    return out.reshape(B, KTOT * FOUT, 1).astype(np.float32)
